# revision 2
# baseline (speedup 1.0000x reference)
"""Boundary-smoothing masked-BCE kernel for Trainium2 (8 NeuronCores), v5.

Math (reference, SB_SIZE=1, SB_EPSILON=0.1):
    P = (target==1), M = (mask==1), xm = x*M
    num = sum(M*softplus(x)) - [T2 + 0.025*(T3 - T4)]
      T2 = sum_{sites s: P=1} xm[s]
      T3 = sum_s sum_{d in 4 dirs} xm[s+d]
      T4 = sum_s xm[s] * nbrM[s],  nbrM = # in-range neighbors with M=1
    out = num / sum(M)

Implementation:
  - Host compacts x[M==1] (pure re-indexing, no value arithmetic; ~50% of
    cells under the triangular span mask) into a fixed [128, F2] fp8 array
    padded with -30; exp(-30) ~ 0 so w = 1+E rounds to exactly 1.0 and
    pads/masked cells contribute exactly 0 to the product tree.
  - Dense masked-softplus on device: E = Exp(xc) on ACT; w = E+1
    (tensor_scalar, 4x DVE mode); product tree w_l*w_r -> v2 -> v4 -> v8
    (2x DVE) so the Ln accumulation pass touches 1/8 of the elements:
      sum ln(1+e^xi) = ln(prod (1+e^xi));  (1+e^5.6)^8 ~ 1e19 < bf16 max.
  - den = sum(M) is an exact integer count; it falls out of the same host
    compaction index and is summed in f64 on the host (the alternative --
    shipping a [1]*n array for the device to add up -- costs ~7us of DMA+PE
    for no additional fidelity).
  - Sparse terms (T2/T3/T4): positives are ~0.2%; host compacts site and
    neighbor values of x and M into small bf16 arrays; device multiplies
    and reduces them.
  - Off-spec inputs (masked count or site count over capacity) fold the
    excess in exactly on the host; never triggered by spec-shaped inputs.
"""
import sys

sys.path.insert(0, "/opt/trn_rl_repo")

import numpy as np
import ml_dtypes

import concourse.bass as bass
import concourse.bacc as bacc
import concourse.tile as tile
import concourse.mybir as mybir
from concourse.bass_utils import run_bass_kernel_spmd

bf16 = mybir.dt.bfloat16
f32 = mybir.dt.float32
fp8 = mybir.dt.float8e4

B, S, L = 16, 256, 24
NCORES = 8
BLOC = B // NCORES            # 2 batches per core
P = 128
NTOT = BLOC * S * S * L       # 3145728 elements per core
F2 = 12368                    # compacted cols (capacity 1583104, ~50.3%)
CAP2 = P * F2
CHUNKS = [1536, 2952, 2952, 2952, 1464, 512]
assert sum(CHUNKS) == F2 and all(c % 8 == 0 for c in CHUNKS)
NCH = len(CHUNKS)
LNGRP = [(0, 4), (4, 6)]      # Ln over chunk groups (fewer ACT instrs)
CAP = 8192                    # max positive sites per core
SC = CAP // P                 # 64 site cols per block
SENT = -30.0                  # pad/mask sentinel: exp(-30) ~ 0

MULT = mybir.AluOpType.mult
ADD = mybir.AluOpType.add
SUB = mybir.AluOpType.subtract
AX = mybir.AxisListType.X
AF = mybir.ActivationFunctionType


def _build_bass():
    nc = bacc.Bacc("TRN2", target_bir_lowering=False)
    xd = nc.declare_dram_parameter("xc", [P, F2], fp8, isOutput=False)
    sd = nc.declare_dram_parameter("sites", [P, 10 * SC], bf16, isOutput=False)
    od = nc.declare_dram_parameter("out", [P, 16], f32, isOutput=True)
    with tile.TileContext(nc) as tc:
        _body(tc, xd, sd, od)
    nc.compile()
    _dedup_act_table_loads(nc)
    return nc


def _dedup_act_table_loads(nc):
    # Exp and Ln both live in natural_log_exp_and_others; bacc's per-function
    # canonical choice alternates sets, paying a table DMA per switch.  Point
    # the first semaphore-free load at the combined set and drop the rest.
    from concourse.hw_specs import get_activation_tables
    names = list(get_activation_tables("gen3").keys())
    target = names.index("natural_log_exp_and_others")
    for bb in nc.main_func.blocks:
        keep = []
        first = True
        for ins in bb.instructions:
            if type(ins).__name__ == "InstLoadActFuncSet":
                si = ins.sync_info
                if si is not None and (si.on_wait or si.on_update):
                    keep.append(ins)
                    continue
                if first:
                    ins.act_func_set_id = target
                    keep.append(ins)
                    first = False
                continue
            keep.append(ins)
        if len(keep) != len(bb.instructions):
            bb.instructions = keep


def _body(tc, xd, sd, od):
    nc = tc.nc
    import contextlib
    ctx = contextlib.ExitStack()
    with ctx:
        const = ctx.enter_context(tc.tile_pool(name="const", bufs=1))
        inx = ctx.enter_context(tc.tile_pool(name="inx", bufs=3))
        ep = ctx.enter_context(tc.tile_pool(name="ep", bufs=2))
        wp = ctx.enter_context(tc.tile_pool(name="wp", bufs=2))
        vp = ctx.enter_context(tc.tile_pool(name="vp", bufs=2))
        lp = ctx.enter_context(tc.tile_pool(name="lp", bufs=2))

        sitet = const.tile([P, 10 * SC], bf16)
        outt = const.tile([P, 16], f32)
        v8all = const.tile([P, F2 // 8], bf16)

        off = np.cumsum([0] + CHUNKS)
        xb = [None] * NCH
        Eb = [None] * NCH

        def load_x(i):
            xb[i] = inx.tile([P, CHUNKS[i]], fp8, tag="xb", name="xb")
            # first two chunks ride the low-latency HWDGE path so the ACT
            # engine starts as early as possible; the rest go through SWDGE
            eng = nc.sync if i < 2 else nc.gpsimd
            eng.dma_start(out=xb[i], in_=xd[:, off[i]:off[i + 1]])

        def emit_exp(i):
            Eb[i] = ep.tile([P, CHUNKS[i]], bf16, tag="E", name="E")
            nc.scalar.activation(Eb[i], xb[i], AF.Exp)

        def emit_tree(i):
            c = CHUNKS[i]
            w = wp.tile([P, c], bf16, tag="w", name="w")
            nc.vector.tensor_scalar_add(w, Eb[i], 1.0)
            h = c // 2
            v2 = vp.tile([P, h], bf16, tag="v2", name="v2")
            nc.vector.tensor_tensor(v2, w[:, 0:h], w[:, h:c], op=MULT)
            q = h // 2
            v4 = vp.tile([P, q], bf16, tag="v4", name="v4")
            nc.vector.tensor_tensor(v4, v2[:, 0:q], v2[:, q:h], op=MULT)
            e = q // 2
            nc.vector.tensor_tensor(v8all[:, off[i] // 8:off[i + 1] // 8],
                                    v4[:, 0:e], v4[:, e:q], op=MULT)

        def emit_ln(g):
            lo, hi = LNGRP[g]
            a, b2 = off[lo] // 8, off[hi] // 8
            sc = lp.tile([P, b2 - a], bf16, tag="lo", name="lo")
            nc.scalar.activation(sc, v8all[:, a:b2], AF.Ln,
                                 accum_out=outt[:, g:g + 1])

        def emit_sites():
            xg = sitet[:, 0:5 * SC]
            mg = sitet[:, 5 * SC:10 * SC]
            xmg = const.tile([P, 5 * SC], bf16)
            nc.vector.tensor_tensor(xmg, xg, mg, op=MULT)
            xm0 = xmg[:, 0:SC]
            nbrM = const.tile([P, SC], bf16)
            nc.vector.tensor_tensor(nbrM, mg[:, SC:2 * SC], mg[:, 2 * SC:3 * SC], op=ADD)
            nc.vector.tensor_tensor(nbrM, nbrM, mg[:, 3 * SC:4 * SC], op=ADD)
            nc.vector.tensor_tensor(nbrM, nbrM, mg[:, 4 * SC:5 * SC], op=ADD)
            sum4 = const.tile([P, SC], bf16)
            nc.vector.tensor_tensor(sum4, xmg[:, SC:2 * SC], xmg[:, 2 * SC:3 * SC], op=ADD)
            nc.vector.tensor_tensor(sum4, sum4, xmg[:, 3 * SC:4 * SC], op=ADD)
            nc.vector.tensor_tensor(sum4, sum4, xmg[:, 4 * SC:5 * SC], op=ADD)
            c1 = const.tile([P, SC], bf16)
            nc.vector.tensor_tensor(c1, xm0, nbrM, op=MULT)
            c2 = const.tile([P, SC], bf16)
            nc.vector.tensor_tensor(c2, sum4, c1, op=SUB)
            nc.vector.tensor_reduce(outt[:, 8:9], xm0, axis=AX, op=ADD)
            nc.vector.tensor_reduce(outt[:, 9:10], c2, axis=AX, op=ADD)

        # software-pipelined emission: Exp(i+1) is queued before the chunk-i
        # DVE tree so ACT never parks; all x loads lead the site load.
        load_x(0)
        load_x(1)
        emit_exp(0)
        lngrp_done = 0
        for i in range(NCH):
            if i + 2 < NCH:
                load_x(i + 2)
            if i == 1:
                nc.gpsimd.dma_start(out=sitet, in_=sd[:, :])
            if i + 1 < NCH:
                emit_exp(i + 1)
            emit_tree(i)
            if i == 1:
                emit_sites()
            while lngrp_done < len(LNGRP) and LNGRP[lngrp_done][1] == i + 1:
                emit_ln(lngrp_done)
                lngrp_done += 1

        nc.sync.dma_start(out=od[:, :], in_=outt)


_BASS_CACHE = {}


def _get_bass():
    if "nc" not in _BASS_CACHE:
        _BASS_CACHE["nc"] = _build_bass()
    return _BASS_CACHE["nc"]


def _prep_core(x, t, m):
    """x,t,m: [BLOC,S,S,L] -> (input map, n_masked, site spill, dense spill)."""
    xf = x.ravel()
    mf = m.ravel()
    sel = np.nonzero(mf != 0)[0]
    nsel = sel.size
    if nsel > CAP2:
        dspill = sel[CAP2:]      # handled exactly on host (off-spec only)
        sel = sel[:CAP2]
    else:
        dspill = None
    xc = np.full(CAP2, SENT, dtype=np.float32)
    xc[:sel.size] = xf[sel]
    xc = xc.astype(ml_dtypes.float8_e4m3)

    sites = np.zeros((10, CAP), dtype=np.float32)
    idx = np.nonzero(t.ravel() == 1.0)[0]
    n = idx.size
    if n > CAP:
        sspill = idx[CAP:]
        idx = idx[:CAP]
        n = CAP
    else:
        sspill = None
    l_s2 = (idx // L) % S
    l_s1 = (idx // (L * S)) % S
    offs = [np.zeros_like(idx),
            np.where(l_s2 < S - 1, idx + L, -1),
            np.where(l_s2 > 0, idx - L, -1),
            np.where(l_s1 < S - 1, idx + L * S, -1),
            np.where(l_s1 > 0, idx - L * S, -1)]
    for k, o in enumerate(offs):
        valid = o >= 0
        ov = o[valid]
        sites[k, :n][valid] = xf[ov]
        sites[5 + k, :n][valid] = mf[ov]
    sitepk = (sites.reshape(10, P, SC).transpose(1, 0, 2)
              .reshape(P, 10 * SC).astype(ml_dtypes.bfloat16))
    inmap = {"xc": xc.reshape(P, F2), "sites": sitepk}
    return inmap, nsel, sspill, dspill


def _host_bracket(xf, mf, idx):
    """Exact f64 bracket contribution for spill sites (off-spec inputs)."""
    l_s2 = (idx // L) % S
    l_s1 = (idx // (L * S)) % S
    xm = xf * mf
    t2 = xm[idx].sum()
    t3 = 0.0
    nbrm = np.zeros(idx.size)
    for o, valid in ((L, l_s2 < S - 1), (-L, l_s2 > 0),
                     (L * S, l_s1 < S - 1), (-L * S, l_s1 > 0)):
        ov = idx[valid] + o
        t3 += xm[ov].sum()
        nbrm[valid] += mf[ov]
    t4 = (xm[idx] * nbrm).sum()
    return t2 + 0.025 * (t3 - t4)


def kernel(predict, target, mask):
    predict = np.asarray(predict, dtype=np.float32)
    target = np.asarray(target, dtype=np.float32)
    mask = np.asarray(mask, dtype=np.int32)

    nc = _get_bass()
    in_maps = []
    nsels = []
    sspills = []
    dspills = []
    for c in range(NCORES):
        b0 = c * BLOC
        im, nsel, ss, ds = _prep_core(predict[b0:b0 + BLOC],
                                      target[b0:b0 + BLOC],
                                      mask[b0:b0 + BLOC])
        in_maps.append(im)
        nsels.append(nsel)
        sspills.append(ss)
        dspills.append(ds)
    res = run_bass_kernel_spmd(nc, in_maps, list(range(NCORES)))

    num = 0.0
    den = 0.0
    for c in range(NCORES):
        o = res.results[c]["out"].astype(np.float64)
        main = o[:, 0:len(LNGRP)].sum()
        bracket = o[:, 8].sum() + 0.025 * o[:, 9].sum()
        b0 = c * BLOC
        if sspills[c] is not None:
            bracket += _host_bracket(
                predict[b0:b0 + BLOC].ravel().astype(np.float64),
                (mask[b0:b0 + BLOC].ravel() != 0).astype(np.float64),
                sspills[c])
        if dspills[c] is not None:
            xs = predict[b0:b0 + BLOC].ravel()[dspills[c]].astype(np.float64)
            main += (np.maximum(xs, 0) + np.log1p(np.exp(-np.abs(xs)))).sum()
        num += main - bracket
        den += nsels[c]
    return np.float32(num / den)
